# revision 6
# baseline (speedup 1.0000x reference)
"""Trainium2 Bass kernel: conv2d(3x3,VALID) + bias -> min over C_out -> tanh(tanh).

Full-input contract: kernel(**inputs) takes the unsharded inputs
  x:           [32, 16, 256, 256] f32
  conv_weight: [64, 16, 3, 3]     f32
  conv_bias:   [64]               f32
and returns [32, 1, 254, 254] f32.

Strategy (data-parallel over batch, 4 images per core on 8 cores):
Row-slab formulation — HBM traffic is ~12 MB/core (vs 60 MB for a
kw-replicated im2col slab; the old kernel was DMA-bound at ~450us).
SBUF holds xr[(t,c), cols] where partition t*16+c (t in 0..6) carries
image row 5g+t of band g as 260 flat columns; partition 112 is ones
(carries the bias through the matmul). Per tile of 128 positions
m = q0..q0+127 in band g, three accumulating matmuls (kw = 0..2):
  ps[m, (r,o)] += xr[:, c0+kw+m].T @ w2[kw]
  w2[kw][(t,c), (r,o)] = W[o, c, t-r, kw]  (0 unless t-r in 0..2)
The horizontal kw tap is just a column offset on the stationary
operand's access pattern; the vertical kh taps come from the t
partition groups serving r = 0..4 packed output rows. PSUM is
[128 positions, 5 rows x 64 ch]; channel-min is a free-dim reduce_min
on DVE, then tanh(tanh()) on ACT. Host drops the 2 garbage cols/rows.
"""

import sys
import types

import numpy as np

# ---------------------------------------------------------------------------
# NTFF profile hook registration (the container's antenv stub lacks
# axon_hooks; registering it enables trace=True for profiling runs).
def _install_axon_hooks():
    try:
        import antenv.axon_hooks  # noqa: F401
        return
    except ImportError:
        pass
    try:
        import antenv
        from trn_agent_boot.trn_boot import _ntff_profile_via_ctypes
    except ImportError:
        return
    mod = types.ModuleType("antenv.axon_hooks")
    _hook = [None]
    mod.set_axon_ntff_profile_hook = lambda h: _hook.__setitem__(0, h)
    mod.get_axon_ntff_profile_hook = lambda: _hook[0]
    sys.modules["antenv.axon_hooks"] = mod
    antenv.axon_hooks = mod
    try:
        mod.set_axon_ntff_profile_hook(
            _ntff_profile_via_ctypes("/opt/axon/libaxon_pjrt.so")
        )
    except Exception:
        pass


_install_axon_hooks()

import concourse.bass as bass  # noqa: E402
import concourse.tile as tile  # noqa: E402
from concourse import bacc, mybir  # noqa: E402
from concourse.bass_utils import run_bass_kernel_spmd  # noqa: E402

N_CORES = 8
IMGS_PER_CORE = 4
C_IN, H, W = 16, 256, 256
C_OUT = 64
OH = OW = 254

R = 5                  # output rows packed per psum tile
T = 7                  # row taps per band (R + 2)
KDIM = T * C_IN + 1    # 113 partitions: 7 rows x 16 ch + ones row
NFREE = R * C_OUT      # 320 psum columns
NBANDS = 51            # bands of 5 output rows -> rows 0..254
BANDCOLS = 260         # flat cols stored per band row chunk
IMGCOLS = NBANDS * BANDCOLS  # 13260 xr cols per image
# tiles per image: 51 bands x 2 column halves; chunked for ACT/store
CHUNKS = [8] * 12 + [6]


def _prep_inputs(x, conv_weight, conv_bias):
    """Host-side packing: row-slab fp16 tensor and matmul weights.

    xr[t*16+c, i, g, m] = x[i, c, (5g+t)*256 + m] (m in 0..259, OOB rows
    zero), row 112 = ones — each partition line is contiguous in DRAM so
    device loads are plain 112-partition DMAs.
    """
    n = x.shape[0]
    xf = x.reshape(n, C_IN, H * W).astype(np.float16)
    xfp = np.zeros((n, C_IN, H * W + 2048), dtype=np.float16)
    xfp[:, :, :H * W] = xf
    xr = np.empty((KDIM, n, NBANDS, BANDCOLS), dtype=np.float16)
    st = xfp.strides
    for t in range(T):
        view = np.lib.stride_tricks.as_strided(
            xfp[:, :, t * W:],
            shape=(n, C_IN, NBANDS, BANDCOLS),
            strides=(st[0], st[1], R * W * st[2], st[2]),
        )
        xr[t * C_IN:(t + 1) * C_IN] = view.transpose(1, 0, 2, 3)
    xr[KDIM - 1] = 1.0

    # w2[(t,c), kw, r, o] = W[o, c, t-r, kw] for t-r in 0..2
    w2 = np.zeros((KDIM, 3, R, C_OUT), dtype=np.float32)
    for kw in range(3):
        for t in range(T):
            for r in range(R):
                kh = t - r
                if 0 <= kh <= 2:
                    w2[t * C_IN:(t + 1) * C_IN, kw, r, :] = (
                        conv_weight[:, :, kh, kw].T
                    )
    w2[KDIM - 1, 0, :, :] = conv_bias[None, :]  # bias via ones row, kw=0 only
    w2 = w2.reshape(KDIM, 3 * NFREE).astype(np.float16)
    return xr, w2


def _build_program():
    nc = bacc.Bacc(
        "TRN2", target_bir_lowering=False, debug=False, num_devices=N_CORES
    )
    f16 = mybir.dt.float16
    f32 = mybir.dt.float32

    x_d = nc.dram_tensor(
        "x", [KDIM, IMGS_PER_CORE * IMGCOLS], f16, kind="ExternalInput"
    )
    w_d = nc.dram_tensor("w", [KDIM, 3 * NFREE], f16, kind="ExternalInput")
    y_d = nc.dram_tensor(
        "y", [IMGS_PER_CORE, NBANDS * R, W], f32, kind="ExternalOutput"
    )

    with tile.TileContext(nc) as tc:
        with (
            tc.tile_pool(name="wpool", bufs=1) as wpool,
            tc.tile_pool(name="slab", bufs=IMGS_PER_CORE) as slab_pool,
            tc.tile_pool(name="stage", bufs=4) as stage_pool,
            tc.tile_pool(name="psum", bufs=4, space="PSUM") as psum_pool,
        ):
            w_t = wpool.tile([KDIM, 3 * NFREE], f16)
            nc.sync.dma_start(w_t[:], w_d[:])

            # Whole-core input resident in SBUF: one slab tile per image so
            # compute on image i only waits for image i's DMA.
            # 112-partition transfers spray across all 16 SDMA engines; the
            # ones row goes separately (113-partition falls to one engine).
            slabs = []
            for i in range(IMGS_PER_CORE):
                s = slab_pool.tile([KDIM, IMGCOLS], f16)
                nc.sync.dma_start(
                    s[0:112, :],
                    x_d[0:112, i * IMGCOLS:(i + 1) * IMGCOLS],
                )
                nc.sync.dma_start(
                    s[112:113, :],
                    x_d[112:113, i * IMGCOLS:(i + 1) * IMGCOLS],
                )
                slabs.append(s)

            for i in range(IMGS_PER_CORE):
                s = slabs[i]
                g0 = 0
                for nb in CHUNKS:
                    ngg = nb // 2  # bands in this chunk
                    mn = stage_pool.tile([128, 2, 4, R], f32, tag="mn")
                    for q in range(2):
                        for gg in range(0, ngg, 2):
                            npair = min(2, ngg - gg)
                            # 2-bank PSUM tile: sub-block sb at elem offset
                            # sb*512 (bank-aligned) so one DVE reduce covers
                            # both tiles, halving reduce-op overhead.
                            ps = psum_pool.tile([128, 2, 512], f32)
                            for sb in range(npair):
                                c0 = (g0 + gg + sb) * BANDCOLS + q * 128
                                for kw in range(3):
                                    nc.tensor.matmul(
                                        ps[:, sb, 0:NFREE],
                                        s[:, c0 + kw:c0 + kw + 128],
                                        w_t[:, kw * NFREE:(kw + 1) * NFREE],
                                        start=(kw == 0),
                                        stop=(kw == 2),
                                    )
                            nc.vector.tensor_reduce(
                                mn[:, q, gg:gg + npair, :],
                                ps[:, 0:npair, 0:NFREE].rearrange(
                                    "p s (r o) -> p s r o", o=C_OUT
                                ),
                                axis=mybir.AxisListType.X,
                                op=mybir.AluOpType.min,
                            )
                    th = stage_pool.tile([128, 2, 4, R], f32, tag="th")
                    nc.scalar.activation(
                        th[:, 0:2, 0:ngg, :], mn[:, 0:2, 0:ngg, :],
                        mybir.ActivationFunctionType.Tanh,
                    )
                    nc.scalar.activation(
                        th[:, 0:2, 0:ngg, :], th[:, 0:2, 0:ngg, :],
                        mybir.ActivationFunctionType.Tanh,
                    )
                    # dst row (5(g0+gg)+v), cols q*128+m; one store per half.
                    # SWDGE queue: keeps output stores off the Sync FIFO
                    # so they never delay the slab DMAs.
                    for q in range(2):
                        dst = y_d[
                            i, R * g0:R * (g0 + ngg), q * 128:(q + 1) * 128
                        ].rearrange("(gg v) m -> m gg v", v=R)
                        nc.gpsimd.dma_start(dst, th[:, q, 0:ngg, :])
                    g0 += ngg
    nc.compile()
    return nc


_NC_CACHE = []


def _get_nc():
    if not _NC_CACHE:
        _NC_CACHE.append(_build_program())
    return _NC_CACHE[0]


def kernel(x, conv_weight, conv_bias, _trace=False):
    x = np.asarray(x, dtype=np.float32)
    conv_weight = np.asarray(conv_weight, dtype=np.float32)
    conv_bias = np.asarray(conv_bias, dtype=np.float32)
    n = x.shape[0]
    assert n == N_CORES * IMGS_PER_CORE

    xr, w2 = _prep_inputs(x, conv_weight, conv_bias)
    nc = _get_nc()
    in_maps = [
        {
            "x": np.ascontiguousarray(
                xr[:, c * IMGS_PER_CORE:(c + 1) * IMGS_PER_CORE]
            ).reshape(KDIM, IMGS_PER_CORE * IMGCOLS),
            "w": w2,
        }
        for c in range(N_CORES)
    ]
    res = run_bass_kernel_spmd(
        nc, in_maps, core_ids=list(range(N_CORES)), trace=_trace
    )
    y = np.concatenate([r["y"] for r in res.results], axis=0)  # [32, 255, 256]
    y = y.reshape(n, 1, 255, 256)[:, :, :OH, :OW]
    out = np.ascontiguousarray(y)
    if _trace:
        kernel._last_result = res
    return out


# revision 13
# speedup vs baseline: 2.4650x; 2.4650x over previous
"""Trainium2 Bass kernel: conv2d(3x3,VALID) + bias -> min over C_out -> tanh(tanh).

Full-input contract: kernel(**inputs) takes the unsharded inputs
  x:           [32, 16, 256, 256] f32
  conv_weight: [64, 16, 3, 3]     f32
  conv_bias:   [64]               f32
and returns [32, 1, 254, 254] f32.

Strategy (data-parallel over batch, 4 images per core on 8 cores):
Row-slab formulation — HBM traffic is ~12 MB/core (vs 60 MB for a
kw-replicated im2col slab; the old kernel was DMA-bound at ~450us).
SBUF holds xr[(t,c), cols] where partition t*16+c (t in 0..6) carries
image row 5g+t of band g as 260 flat columns; partition 112 is ones
(carries the bias through the matmul). Per tile of 128 positions
m = q0..q0+127 in band g, three accumulating matmuls (kw = 0..2):
  ps[m, (r,o)] += xr[:, c0+kw+m].T @ w2[kw]
  w2[kw][(t,c), (r,o)] = W[o, c, t-r, kw]  (0 unless t-r in 0..2)
The horizontal kw tap is just a column offset on the stationary
operand's access pattern; the vertical kh taps come from the t
partition groups serving r = 0..4 packed output rows. PSUM is
[128 positions, 5 rows x 64 ch]; channel-min is a free-dim reduce_min
on DVE, then tanh(tanh()) on ACT. Host drops the 2 garbage cols/rows.
"""

import sys
import types

import numpy as np

# ---------------------------------------------------------------------------
# NTFF profile hook registration (the container's antenv stub lacks
# axon_hooks; registering it enables trace=True for profiling runs).
def _install_axon_hooks():
    try:
        import antenv.axon_hooks  # noqa: F401
        return
    except ImportError:
        pass
    try:
        import antenv
        from trn_agent_boot.trn_boot import _ntff_profile_via_ctypes
    except ImportError:
        return
    mod = types.ModuleType("antenv.axon_hooks")
    _hook = [None]
    mod.set_axon_ntff_profile_hook = lambda h: _hook.__setitem__(0, h)
    mod.get_axon_ntff_profile_hook = lambda: _hook[0]
    sys.modules["antenv.axon_hooks"] = mod
    antenv.axon_hooks = mod
    try:
        mod.set_axon_ntff_profile_hook(
            _ntff_profile_via_ctypes("/opt/axon/libaxon_pjrt.so")
        )
    except Exception:
        pass


_install_axon_hooks()

import concourse.bass as bass  # noqa: E402
import concourse.tile as tile  # noqa: E402
from concourse import bacc, mybir  # noqa: E402
from concourse.bass_utils import run_bass_kernel_spmd  # noqa: E402

N_CORES = 8
IMGS_PER_CORE = 4
C_IN, H, W = 16, 256, 256
C_OUT = 64
OH = OW = 254

R = 5                  # output rows packed per psum tile
T = 7                  # row taps per band (R + 2)
KDIM = T * C_IN + 1    # 113 partitions: 7 rows x 16 ch + ones row
NFREE = R * C_OUT      # 320 psum columns
NBANDS = 51            # bands of 5 output rows -> rows 0..254
BANDCOLS = 260         # flat cols stored per band row chunk
IMGCOLS = NBANDS * BANDCOLS  # 13260 xr cols per image
# tiles per image: 51 bands x 2 column halves; chunked for ACT/store
CHUNKS = [8] * 12 + [6]


def _prep_inputs(x, conv_weight, conv_bias):
    """Host-side packing: row-slab fp16 tensor and matmul weights.

    xr[t*16+c, i, g, m] = x[i, c, (5g+t)*256 + m] (m in 0..259, OOB rows
    zero), row 112 = ones — each partition line is contiguous in DRAM so
    device loads are plain 112-partition DMAs.
    """
    n = x.shape[0]
    xf = x.reshape(n, C_IN, H * W).astype(np.float16)
    xfp = np.zeros((n, C_IN, H * W + 2048), dtype=np.float16)
    xfp[:, :, :H * W] = xf
    xr = np.empty((KDIM, n, NBANDS, BANDCOLS), dtype=np.float16)
    st = xfp.strides
    for t in range(T):
        view = np.lib.stride_tricks.as_strided(
            xfp[:, :, t * W:],
            shape=(n, C_IN, NBANDS, BANDCOLS),
            strides=(st[0], st[1], R * W * st[2], st[2]),
        )
        xr[t * C_IN:(t + 1) * C_IN] = view.transpose(1, 0, 2, 3)
    xr[KDIM - 1] = 1.0

    # w2[(t,c), kw, r, o] = W[o, c, t-r, kw] for t-r in 0..2
    w2 = np.zeros((KDIM, 3, R, C_OUT), dtype=np.float32)
    for kw in range(3):
        for t in range(T):
            for r in range(R):
                kh = t - r
                if 0 <= kh <= 2:
                    w2[t * C_IN:(t + 1) * C_IN, kw, r, :] = (
                        conv_weight[:, :, kh, kw].T
                    )
    w2[KDIM - 1, 0, :, :] = conv_bias[None, :]  # bias via ones row, kw=0 only
    w2 = w2.reshape(KDIM, 3 * NFREE).astype(np.float16)
    return xr, w2


def _build_program():
    nc = bacc.Bacc(
        "TRN2", target_bir_lowering=False, debug=False, num_devices=N_CORES
    )
    f16 = mybir.dt.float16
    f32 = mybir.dt.float32

    x_d = nc.dram_tensor(
        "x", [KDIM, IMGS_PER_CORE * IMGCOLS], f16, kind="ExternalInput"
    )
    w_d = nc.dram_tensor("w", [KDIM, 3 * NFREE], f16, kind="ExternalInput")
    # Output stored transposed [img, col, row] so each store's innermost
    # dim (5 consecutive rows of one column) is DRAM-contiguous — 20-byte
    # descriptor runs instead of scattered 4-byte writes. Host transposes.
    y_d = nc.dram_tensor(
        "y", [IMGS_PER_CORE, W, NBANDS * R], f32, kind="ExternalOutput"
    )

    with tile.TileContext(nc) as tc:
        with (
            tc.tile_pool(name="wpool", bufs=1) as wpool,
            tc.tile_pool(name="slab", bufs=1) as slab_pool,
            tc.tile_pool(name="stage", bufs=4) as stage_pool,
            tc.tile_pool(name="psum", bufs=4, space="PSUM") as psum_pool,
        ):
            w_t = wpool.tile([KDIM, 3 * NFREE], f16)
            nc.sync.dma_start(w_t[:], w_d[:])

            # Whole-core input resident in SBUF: one slab tile per image,
            # loaded in 4 band-aligned column chunks so the first matmuls
            # only wait for ~1/4 image instead of a full image.
            # 112-partition transfers spray across all 16 SDMA engines; the
            # ones row goes separately (113-partition falls to one engine).
            slabs = [
                slab_pool.tile([KDIM, IMGCOLS], f16, name=f"slab{i}")
                for i in range(IMGS_PER_CORE)
            ]
            bchunks = [13, 13, 13, 12]
            for i in range(IMGS_PER_CORE):
                b0 = 0
                for nbl in bchunks:
                    lo = b0 * BANDCOLS
                    hi = (b0 + nbl) * BANDCOLS
                    nc.sync.dma_start(
                        slabs[i][112:113, lo:hi],
                        x_d[112:113, i * IMGCOLS + lo:i * IMGCOLS + hi],
                    )
                    nc.sync.dma_start(
                        slabs[i][0:112, lo:hi],
                        x_d[0:112, i * IMGCOLS + lo:i * IMGCOLS + hi],
                    )
                    b0 += nbl

            for i in range(IMGS_PER_CORE):
                s = slabs[i]
                g0 = 0
                for nb in CHUNKS:
                    ngg = nb // 2  # bands in this chunk
                    mn = stage_pool.tile([128, 2, 4, R], f32, tag="mn")
                    for q in range(2):
                        for gg in range(0, ngg, 2):
                            npair = min(2, ngg - gg)
                            # 2-bank PSUM tile: sub-block sb at elem offset
                            # sb*512 (bank-aligned) so one DVE reduce covers
                            # both tiles, halving reduce-op overhead.
                            ps = psum_pool.tile([128, 2, 512], f32)
                            for sb in range(npair):
                                c0 = (g0 + gg + sb) * BANDCOLS + q * 128
                                for kw in range(3):
                                    nc.tensor.matmul(
                                        ps[:, sb, 0:NFREE],
                                        s[:, c0 + kw:c0 + kw + 128],
                                        w_t[:, kw * NFREE:(kw + 1) * NFREE],
                                        start=(kw == 0),
                                        stop=(kw == 2),
                                    )
                            nc.vector.tensor_reduce(
                                mn[:, q, gg:gg + npair, :],
                                ps[:, 0:npair, 0:NFREE].rearrange(
                                    "p s (r o) -> p s r o", o=C_OUT
                                ),
                                axis=mybir.AxisListType.X,
                                op=mybir.AluOpType.min,
                            )
                    th = stage_pool.tile([128, 2, 4, R], f32, tag="th")
                    nc.scalar.activation(
                        th[:, 0:2, 0:ngg, :], mn[:, 0:2, 0:ngg, :],
                        mybir.ActivationFunctionType.Tanh,
                    )
                    nc.scalar.activation(
                        th[:, 0:2, 0:ngg, :], th[:, 0:2, 0:ngg, :],
                        mybir.ActivationFunctionType.Tanh,
                    )
                    # dst col q*128+m, rows 5(g0+gg)+v; one store per half.
                    # Scalar HWDGE queue: keeps stores off the Sync FIFO
                    # (slab loads) without SWDGE descriptor overhead.
                    for q in range(2):
                        dst = y_d[
                            i, q * 128:(q + 1) * 128, R * g0:R * (g0 + ngg)
                        ].rearrange("m (gg v) -> m gg v", v=R)
                        nc.scalar.dma_start(dst, th[:, q, 0:ngg, :])
                    g0 += ngg
    nc.compile()
    return nc


_NC_CACHE = []


def _get_nc():
    if not _NC_CACHE:
        _NC_CACHE.append(_build_program())
    return _NC_CACHE[0]


def kernel(x, conv_weight, conv_bias, _trace=False):
    x = np.asarray(x, dtype=np.float32)
    conv_weight = np.asarray(conv_weight, dtype=np.float32)
    conv_bias = np.asarray(conv_bias, dtype=np.float32)
    n = x.shape[0]
    assert n == N_CORES * IMGS_PER_CORE

    xr, w2 = _prep_inputs(x, conv_weight, conv_bias)
    nc = _get_nc()
    in_maps = [
        {
            "x": np.ascontiguousarray(
                xr[:, c * IMGS_PER_CORE:(c + 1) * IMGS_PER_CORE]
            ).reshape(KDIM, IMGS_PER_CORE * IMGCOLS),
            "w": w2,
        }
        for c in range(N_CORES)
    ]
    res = run_bass_kernel_spmd(
        nc, in_maps, core_ids=list(range(N_CORES)), trace=_trace
    )
    # device emits [img, col(256), row(255)]; transpose back to row-major
    y = np.concatenate([r["y"] for r in res.results], axis=0)  # [32, 256, 255]
    y = y.transpose(0, 2, 1).reshape(n, 1, 255, 256)[:, :, :OH, :OW]
    out = np.ascontiguousarray(y)
    if _trace:
        kernel._last_result = res
    return out
